# revision 9
# baseline (speedup 1.0000x reference)
"""Trainium2 Bass kernel for label-attention:
    scores = einsum('bse,le->bls', x, U) ; alpha = softmax(scores, axis=-1)
    out    = einsum('bls,bse->ble', alpha, x)
Returns (out, alpha) like the reference.

Strategy: data-parallel over batch B=8 -> one batch element per NeuronCore.
Per core, compute scoresT [S, L] = xT.T @ UT with f32r matmuls (full-rate PE),
exp on the scalar engine (no max subtraction needed: scores ~ N(0,1)).
The vector engine accumulates acc = sum_s expT while exp streams; Z comes
from one ones.T @ acc matmul as a [1, W] row (for the alpha normalizer)
plus four acc-slice.T @ ones matmuls as [128, 1] columns (for per-partition
out-row scaling). matmul2 runs on the *unnormalized* expT (no dependency
on the normalize chain); out rows are scaled by recip-Z on the vector
engine. alpha = expT * bcast(recipZ) is written into small quarter
buffers and streamed to DRAM piecewise, fully off the critical path.
Host returns alphaT transposed as a view.
"""
import os
import sys
import types

sys.path.insert(0, "/opt/trn_rl_repo")

import numpy as np

B, S, E, L = 8, 2048, 512, 8929
LPAD = 8960  # 70 * 128
# matmul1 N-chunks over labels: 17 x 512 + 1 x 256 (labels 8704..8959 padded)
CHUNKS = [(c * 512, 512) for c in range(17)] + [(8704, 256)]
NCORES = 8

LAST_EXEC_TIME_NS = None
LAST_TRACE_PATH = None

_cache = {}


def _install_trace_shim():
    """antenv.axon_hooks is missing in this image; reconstruct it so
    run_bass_kernel_spmd(trace=True) can capture NTFF profiles."""
    if "antenv.axon_hooks" in sys.modules:
        return
    try:
        import antenv  # noqa: F401
        from trn_agent_boot.trn_boot import _ntff_profile_via_ctypes

        hook = _ntff_profile_via_ctypes("/opt/axon/libaxon_pjrt.so")
    except Exception:
        hook = None
    mod = types.ModuleType("antenv.axon_hooks")
    mod.get_axon_ntff_profile_hook = lambda: hook
    mod.set_axon_ntff_profile_hook = lambda h: None
    sys.modules["antenv.axon_hooks"] = mod


def _build():
    import concourse.bass as bass  # noqa: F401
    import concourse.mybir as mybir
    import concourse.tile as tile
    from concourse import bacc

    f32 = mybir.dt.float32
    f32r = mybir.dt.float32r
    Exp = mybir.ActivationFunctionType.Exp
    Copy = mybir.ActivationFunctionType.Copy

    nc = bacc.Bacc(
        "TRN2",
        target_bir_lowering=False,
        debug=False,
        enable_asserts=False,
        num_devices=NCORES,
    )

    xT_d = nc.declare_dram_parameter("xT", [E, S], f32, isOutput=False)
    x_d = nc.declare_dram_parameter("x", [S, E], f32, isOutput=False)
    UT_d = nc.declare_dram_parameter("UT", [E, LPAD], f32, isOutput=False)
    alphaT_d = nc.declare_dram_parameter("alphaT", [S, L], f32, isOutput=True)
    out_d = nc.declare_dram_parameter("out", [L, E], f32, isOutput=True)

    with tile.TileContext(nc) as tc:
        with (
            nc.allow_low_precision(
                reason="f32r tiles hold pre-rounded fp32; quantization ~1e-4"
            ),
            tc.tile_pool(name="const", bufs=1) as const_pool,
            tc.tile_pool(name="ut", bufs=2) as ut_pool,
            tc.tile_pool(name="exp", bufs=2) as exp_pool,
            tc.tile_pool(name="alph", bufs=2) as alph_pool,
            tc.tile_pool(name="outsb", bufs=2) as out_pool,
            tc.tile_pool(name="acc", bufs=2) as acc_pool,
            tc.tile_pool(name="small", bufs=2) as small_pool,
            tc.tile_pool(name="ps", bufs=3, space="PSUM") as ps_pool,
            tc.tile_pool(name="po", bufs=2, space="PSUM") as po_pool,
            tc.tile_pool(name="pzr", bufs=1, space="PSUM") as pzr_pool,
            tc.tile_pool(name="pzc", bufs=1, space="PSUM") as pzc_pool,
            tc.tile_pool(name="pb", bufs=1, space="PSUM") as pb_pool,
        ):
            # persistent inputs
            xTs = const_pool.tile([128, 4 * S], f32r)  # e-tiles of x[b].T
            nc.sync.dma_start(
                xTs[:].rearrange("p (t s) -> p t s", t=4),
                xT_d[:].bitcast(f32r).rearrange("(t p) s -> p t s", p=128),
            )
            xs = const_pool.tile([128, 16 * E], f32r)  # s-tiles of x[b]
            nc.sync.dma_start(
                xs[:].rearrange("p (t e) -> p t e", t=16),
                x_d[:].bitcast(f32r).rearrange("(t p) e -> p t e", p=128),
            )
            onesf = const_pool.tile([128, 2], f32)
            nc.gpsimd.memset(onesf[:], 1.0)
            ones_col = const_pool.tile([128, 2], f32r)
            nc.scalar.activation(ones_col[:], onesf[:], Copy)
            onesrf = const_pool.tile([1, 128], f32)
            nc.gpsimd.memset(onesrf[:], 1.0)
            ones_row = const_pool.tile([1, 128], f32r)
            nc.scalar.activation(ones_row[:], onesrf[:], Copy)

            exp_tiles = {}
            acc_tiles = {}
            z_state = {}

            def emit_A(c):
                """matmul1 (scoresT chunk) + exp + DVE row-accumulate."""
                l0, W = CHUNKS[c]
                ut = ut_pool.tile([128, 4 * 512], f32r, tag="ut")
                nc.sync.dma_start(
                    ut[:, : 4 * W].rearrange("p (t w) -> p t w", t=4),
                    UT_d[:, l0 : l0 + W]
                    .bitcast(f32r)
                    .rearrange("(t p) w -> p t w", p=128),
                )
                expT = exp_pool.tile([128, 16 * 512], f32r, tag="exp")
                acc = acc_pool.tile([128, 512], f32r, tag="acc")
                exp_tiles[c] = expT
                acc_tiles[c] = acc
                for s in range(16):
                    ps = ps_pool.tile([128, 512], f32, tag="ps")
                    for e in range(4):
                        nc.tensor.matmul(
                            ps[:, :W],
                            xTs[:, e * S + s * 128 : e * S + (s + 1) * 128],
                            ut[:, e * W : (e + 1) * W],
                            start=(e == 0),
                            stop=(e == 3),
                        )
                    nc.scalar.activation(expT[:, s * W : (s + 1) * W], ps[:, :W], Exp)
                    if s == 0:
                        nc.gpsimd.tensor_copy(acc[:, :W], expT[:, 0:W].bitcast(f32))
                    else:
                        nc.gpsimd.tensor_add(
                            acc[:, :W],
                            acc[:, :W].bitcast(f32),
                            expT[:, s * W : (s + 1) * W].bitcast(f32),
                        )

            def emit_Zhead(c):
                """Z row (for alpha) + Z columns (for out rows) from acc."""
                l0, W = CHUNKS[c]
                nj = W // 128
                acc = acc_tiles.pop(c)
                pzr = pzr_pool.tile([1, 512], f32, tag="pzr")
                nc.tensor.matmul(
                    pzr[0:1, :W], ones_col[:, 0:1], acc[:, :W], start=True, stop=True
                )
                rrow = small_pool.tile([1, 512], f32r, tag="rrow")
                nc.vector.reciprocal(rrow[0:1, :W], pzr[0:1, :W])
                pzc = pzc_pool.tile([128, 2], f32, tag="pzc")
                rcol = small_pool.tile([128, 4], f32, tag="rcol")
                for j in range(nj):
                    nc.tensor.matmul(
                        pzc[:],
                        acc[:, j * 128 : (j + 1) * 128],
                        ones_col[:],
                        start=True,
                        stop=True,
                    )
                    nc.vector.reciprocal(rcol[:, j : j + 1], pzc[:, 0:1])
                z_state[c] = (rrow, rcol)

            def emit_B(c):
                """matmul2 on unnormalized expT + out scaling + alpha
                normalize into quarter buffers + stores."""
                l0, W = CHUNKS[c]
                nj = W // 128
                Wv = min(W, L - l0)
                expT = exp_tiles.pop(c)
                rrow, rcol = z_state.pop(c)
                outsb = out_pool.tile([128, 4 * E], f32, tag="outsb")
                pb = pb_pool.tile([128, 512], f32, tag="pb")

                def norm_piece(piece):
                    alph = alph_pool.tile([128, 4 * 512], f32, tag="alph")
                    for q in range(4):
                        s = piece * 4 + q
                        nc.vector.tensor_mul(
                            alph[:, q * W : (q + 1) * W],
                            expT[:, s * W : (s + 1) * W].bitcast(f32),
                            pb[:, :W],
                        )
                    nc.sync.dma_start(
                        alphaT_d[
                            512 * piece : 512 * (piece + 1), l0 : l0 + Wv
                        ].rearrange("(t p) w -> p t w", p=128),
                        alph[:, : 4 * W].rearrange("p (t w) -> p t w", t=4)[
                            :, :, 0:Wv
                        ],
                    )

                for j in range(nj):
                    po = po_pool.tile([128, E], f32, tag="po")
                    for s in range(16):
                        lhsT = expT[:, s * W + j * 128 : s * W + (j + 1) * 128]
                        nc.tensor.matmul(
                            po[:],
                            lhsT,
                            xs[:, s * E : (s + 1) * E],
                            start=(s == 0),
                            stop=(s == 15),
                        )
                    nc.vector.tensor_scalar_mul(
                        outsb[:, j * E : (j + 1) * E], po[:], rcol[:, j : j + 1]
                    )
                    if j == 0:
                        # broadcast recip-Z row across partitions; recip is
                        # long done by the time the first po group retires
                        nc.tensor.matmul(
                            pb[:, :W],
                            ones_row[0:1, 0:128],
                            rrow[0:1, :W],
                            start=True,
                            stop=True,
                        )
                    else:
                        norm_piece(j - 1)
                for piece in range(nj - 1, 4):
                    norm_piece(piece)
                # store out chunk
                if l0 + nj * 128 <= L:
                    nc.sync.dma_start(
                        out_d[l0 : l0 + nj * 128, :].rearrange(
                            "(j p) e -> p j e", p=128
                        ),
                        outsb[:, : nj * E].rearrange("p (j e) -> p j e", j=nj),
                    )
                else:
                    for j in range(nj):
                        r0 = l0 + j * 128
                        nr = min(128, L - r0)
                        if nr <= 0:
                            break
                        nc.sync.dma_start(
                            out_d[r0 : r0 + nr, :],
                            outsb[0:nr, j * E : (j + 1) * E],
                        )

            n = len(CHUNKS)
            emit_A(0)
            for c in range(1, n):
                emit_A(c)
                emit_Zhead(c - 1)
                emit_B(c - 1)
            emit_Zhead(n - 1)
            emit_B(n - 1)

    nc.compile()
    return nc


def kernel(x, U_weight):
    global LAST_EXEC_TIME_NS, LAST_TRACE_PATH
    _install_trace_shim()
    from concourse import bass_utils
    from concourse.bass_utils import run_bass_kernel_spmd

    bass_utils.upload_artifacts = lambda d: d  # no S3 in this container

    if "nc" not in _cache:
        _cache["nc"] = _build()
    nc = _cache["nc"]

    x = np.asarray(x, dtype=np.float32)
    U_weight = np.asarray(U_weight, dtype=np.float32)

    UTp = np.zeros((E, LPAD), dtype=np.float32)
    UTp[:, :L] = U_weight.T

    in_maps = []
    for b in range(B):
        in_maps.append(
            {
                "xT": np.ascontiguousarray(x[b].T),
                "x": np.ascontiguousarray(x[b]),
                "UT": UTp,
            }
        )

    trace = bool(os.environ.get("TRN_KERNEL_TRACE"))
    res = run_bass_kernel_spmd(nc, in_maps, list(range(NCORES)), trace=trace)
    LAST_EXEC_TIME_NS = res.exec_time_ns
    if res.instructions_and_trace is not None:
        LAST_TRACE_PATH = res.instructions_and_trace[1]

    out = np.stack([res.results[b]["out"] for b in range(B)])
    alphaT = np.stack([res.results[b]["alphaT"] for b in range(B)])
    alpha = alphaT.transpose(0, 2, 1)  # [B, L, S] view, no copy
    return out, alpha


# revision 10
# speedup vs baseline: 1.0896x; 1.0896x over previous
"""Trainium2 Bass kernel for label-attention:
    scores = einsum('bse,le->bls', x, U) ; alpha = softmax(scores, axis=-1)
    out    = einsum('bls,bse->ble', alpha, x)
Returns (out, alpha) like the reference.

Strategy: data-parallel over batch B=8 -> one batch element per NeuronCore.
Per core, compute scoresT [S, L] = xT.T @ UT with f32r matmuls (full-rate PE),
exp on the scalar engine (no max subtraction needed: scores ~ N(0,1)).
Z is formed without burning PE slots: the vector engine accumulates
acc = sum_s expT s-tiles while exp streams, then a single ones.T @ acc
matmul reduces over partitions giving Z as a [1, W] row; reciprocal +
K=1 broadcast matmul turn it into a [128, W] recip tile. alpha is
normalized in place (f32r), so matmul2 (alphaT.T @ x) directly yields the
normalized out rows. alphaT [S, L] goes to DRAM; host returns the
transposed view.
"""
import os
import sys
import types

sys.path.insert(0, "/opt/trn_rl_repo")

import numpy as np

B, S, E, L = 8, 2048, 512, 8929
LPAD = 8960  # 70 * 128
# matmul1 N-chunks over labels: 17 x 512 + 1 x 256 (labels 8704..8959 padded)
CHUNKS = [(c * 512, 512) for c in range(17)] + [(8704, 256)]
NCORES = 8

LAST_EXEC_TIME_NS = None
LAST_TRACE_PATH = None

_cache = {}


def _install_trace_shim():
    """antenv.axon_hooks is missing in this image; reconstruct it so
    run_bass_kernel_spmd(trace=True) can capture NTFF profiles."""
    if "antenv.axon_hooks" in sys.modules:
        return
    try:
        import antenv  # noqa: F401
        from trn_agent_boot.trn_boot import _ntff_profile_via_ctypes

        hook = _ntff_profile_via_ctypes("/opt/axon/libaxon_pjrt.so")
    except Exception:
        hook = None
    mod = types.ModuleType("antenv.axon_hooks")
    mod.get_axon_ntff_profile_hook = lambda: hook
    mod.set_axon_ntff_profile_hook = lambda h: None
    sys.modules["antenv.axon_hooks"] = mod


def _build():
    import concourse.bass as bass  # noqa: F401
    import concourse.mybir as mybir
    import concourse.tile as tile
    from concourse import bacc

    f32 = mybir.dt.float32
    f32r = mybir.dt.float32r
    Exp = mybir.ActivationFunctionType.Exp
    Copy = mybir.ActivationFunctionType.Copy

    nc = bacc.Bacc(
        "TRN2",
        target_bir_lowering=False,
        debug=False,
        enable_asserts=False,
        num_devices=NCORES,
    )

    xT_d = nc.declare_dram_parameter("xT", [E, S], f32, isOutput=False)
    x_d = nc.declare_dram_parameter("x", [S, E], f32, isOutput=False)
    UT_d = nc.declare_dram_parameter("UT", [E, LPAD], f32, isOutput=False)
    alphaT_d = nc.declare_dram_parameter("alphaT", [S, L], f32, isOutput=True)
    out_d = nc.declare_dram_parameter("out", [L, E], f32, isOutput=True)

    with tile.TileContext(nc) as tc:
        with (
            nc.allow_low_precision(
                reason="f32r tiles hold pre-rounded fp32; quantization ~1e-4"
            ),
            tc.tile_pool(name="const", bufs=1) as const_pool,
            tc.tile_pool(name="ut", bufs=2) as ut_pool,
            tc.tile_pool(name="exp", bufs=2) as exp_pool,
            tc.tile_pool(name="outsb", bufs=2) as out_pool,
            tc.tile_pool(name="acc", bufs=2) as acc_pool,
            tc.tile_pool(name="small", bufs=2) as small_pool,
            tc.tile_pool(name="ps", bufs=3, space="PSUM") as ps_pool,
            tc.tile_pool(name="po", bufs=2, space="PSUM") as po_pool,
            tc.tile_pool(name="pzr", bufs=1, space="PSUM") as pzr_pool,
            tc.tile_pool(name="pb", bufs=2, space="PSUM") as pb_pool,
        ):
            # persistent inputs
            xTs = const_pool.tile([128, 4 * S], f32r)  # e-tiles of x[b].T
            nc.sync.dma_start(
                xTs[:].rearrange("p (t s) -> p t s", t=4),
                xT_d[:].bitcast(f32r).rearrange("(t p) s -> p t s", p=128),
            )
            xs = const_pool.tile([128, 16 * E], f32r)  # s-tiles of x[b]
            nc.sync.dma_start(
                xs[:].rearrange("p (t e) -> p t e", t=16),
                x_d[:].bitcast(f32r).rearrange("(t p) e -> p t e", p=128),
            )
            onesf = const_pool.tile([128, 2], f32)
            nc.gpsimd.memset(onesf[:], 1.0)
            ones_col = const_pool.tile([128, 2], f32r)
            nc.scalar.activation(ones_col[:], onesf[:], Copy)
            onesrf = const_pool.tile([1, 128], f32)
            nc.gpsimd.memset(onesrf[:], 1.0)
            ones_row = const_pool.tile([1, 128], f32r)
            nc.scalar.activation(ones_row[:], onesrf[:], Copy)

            exp_tiles = {}
            acc_tiles = {}
            zr_tiles = {}

            def emit_A(c, ztail_for=None):
                """matmul1 (scoresT chunk) + exp + DVE row-accumulate.
                After the first s-group, emit the previous chunk's Z tail
                so its PE ops hide inside this chunk's matmul stream."""
                l0, W = CHUNKS[c]
                ut = ut_pool.tile([128, 4 * 512], f32r, tag="ut")
                nc.sync.dma_start(
                    ut[:, : 4 * W].rearrange("p (t w) -> p t w", t=4),
                    UT_d[:, l0 : l0 + W]
                    .bitcast(f32r)
                    .rearrange("(t p) w -> p t w", p=128),
                )
                expT = exp_pool.tile([128, 16 * 512], f32r, tag="exp")
                acc = acc_pool.tile([128, 512], f32r, tag="acc")
                exp_tiles[c] = expT
                acc_tiles[c] = acc
                for s in range(16):
                    ps = ps_pool.tile([128, 512], f32, tag="ps")
                    for e in range(4):
                        nc.tensor.matmul(
                            ps[:, :W],
                            xTs[:, e * S + s * 128 : e * S + (s + 1) * 128],
                            ut[:, e * W : (e + 1) * W],
                            start=(e == 0),
                            stop=(e == 3),
                        )
                    nc.scalar.activation(expT[:, s * W : (s + 1) * W], ps[:, :W], Exp)
                    if s == 0:
                        nc.vector.tensor_copy(
                            acc[:, :W], expT[:, 0:W].bitcast(f32)
                        )
                    elif s < 14:
                        nc.vector.tensor_add(
                            acc[:, :W],
                            acc[:, :W].bitcast(f32),
                            expT[:, s * W : (s + 1) * W].bitcast(f32),
                        )
                    if s == 0 and ztail_for is not None:
                        emit_Z(ztail_for)

            def emit_Z(c):
                """Z row-reduce + reciprocal + broadcast + in-place normalize
                of alphaT + alphaT store."""
                l0, W = CHUNKS[c]
                Wv = min(W, L - l0)
                expT = exp_tiles[c]
                acc = acc_tiles.pop(c)
                pzr = pzr_pool.tile([1, 512], f32, tag="pzr")
                nc.tensor.matmul(
                    pzr[0:1, :W], ones_col[:, 0:1], acc[:, :W],
                    start=True, stop=False, skip_group_check=True,
                )
                for s in (14, 15):
                    nc.tensor.matmul(
                        pzr[0:1, :W],
                        ones_col[:, 0:1],
                        expT[:, s * W : (s + 1) * W],
                        start=False, stop=(s == 15), skip_group_check=True,
                    )
                rrow = small_pool.tile([1, 512], f32r, tag="rrow")
                nc.vector.reciprocal(rrow[0:1, :W], pzr[0:1, :W])
                pb = pb_pool.tile([128, 512], f32, tag="pb")
                nc.tensor.matmul(
                    pb[:, :W],
                    ones_row[0:1, 0:128],
                    rrow[0:1, :W],
                    start=True,
                    stop=True,
                )
                zr_tiles[c] = pb
                for s in range(16):
                    sl = expT[:, s * W : s * W + W]
                    nc.vector.tensor_mul(sl, sl.bitcast(f32), pb[:, :W])
                nc.sync.dma_start(
                    alphaT_d[:, l0 : l0 + Wv].rearrange("(t p) w -> p t w", p=128),
                    expT[:, : 16 * W]
                    .bitcast(f32)
                    .rearrange("p (t w) -> p t w", t=16)[:, :, 0:Wv],
                )

            def emit_B(c):
                """matmul2 on normalized alphaT -> out rows + store."""
                l0, W = CHUNKS[c]
                nj = W // 128
                expT = exp_tiles.pop(c)
                zr_tiles.pop(c, None)
                outsb = out_pool.tile([128, 4 * E], f32, tag="outsb")
                for j in range(nj):
                    po = po_pool.tile([128, E], f32, tag="po")
                    for s in range(16):
                        lhsT = expT[:, s * W + j * 128 : s * W + (j + 1) * 128]
                        nc.tensor.matmul(
                            po[:],
                            lhsT,
                            xs[:, s * E : (s + 1) * E],
                            start=(s == 0),
                            stop=(s == 15),
                        )
                    nc.vector.tensor_copy(outsb[:, j * E : (j + 1) * E], po[:])
                if l0 + nj * 128 <= L:
                    nc.sync.dma_start(
                        out_d[l0 : l0 + nj * 128, :].rearrange(
                            "(j p) e -> p j e", p=128
                        ),
                        outsb[:, : nj * E].rearrange("p (j e) -> p j e", j=nj),
                    )
                else:
                    for j in range(nj):
                        r0 = l0 + j * 128
                        nr = min(128, L - r0)
                        if nr <= 0:
                            break
                        nc.sync.dma_start(
                            out_d[r0 : r0 + nr, :],
                            outsb[0:nr, j * E : (j + 1) * E],
                        )

            n = len(CHUNKS)
            emit_A(0)
            for c in range(1, n):
                emit_A(c, ztail_for=c - 1)
                emit_B(c - 1)
            emit_Z(n - 1)
            emit_B(n - 1)

    nc.compile()
    return nc


def kernel(x, U_weight):
    global LAST_EXEC_TIME_NS, LAST_TRACE_PATH
    _install_trace_shim()
    from concourse import bass_utils
    from concourse.bass_utils import run_bass_kernel_spmd

    bass_utils.upload_artifacts = lambda d: d  # no S3 in this container

    if "nc" not in _cache:
        _cache["nc"] = _build()
    nc = _cache["nc"]

    x = np.asarray(x, dtype=np.float32)
    U_weight = np.asarray(U_weight, dtype=np.float32)

    UTp = np.zeros((E, LPAD), dtype=np.float32)
    UTp[:, :L] = U_weight.T

    in_maps = []
    for b in range(B):
        in_maps.append(
            {
                "xT": np.ascontiguousarray(x[b].T),
                "x": np.ascontiguousarray(x[b]),
                "UT": UTp,
            }
        )

    trace = bool(os.environ.get("TRN_KERNEL_TRACE"))
    res = run_bass_kernel_spmd(nc, in_maps, list(range(NCORES)), trace=trace)
    LAST_EXEC_TIME_NS = res.exec_time_ns
    if res.instructions_and_trace is not None:
        LAST_TRACE_PATH = res.instructions_and_trace[1]

    out = np.stack([res.results[b]["out"] for b in range(B)])
    alphaT = np.stack([res.results[b]["alphaT"] for b in range(B)])
    alpha = alphaT.transpose(0, 2, 1)  # [B, L, S] view, no copy
    return out, alpha
